# revision 30
# baseline (speedup 1.0000x reference)
"""Trainium2 Bass/Tile kernel for CrossModalMultiHeadAttention.

Reference computation (per batch element b, eval mode):
    v_norm = LN(v) ; l_norm = LN(l)
    updated_v = MHA(q=v_norm, kv=l_norm; W*_v2l)   # v attends to l
    updated_l = MHA(q=l_norm, kv=v_norm; W*_l2v)   # l attends to v
    v_out = v + gamma_v * updated_v
    l_out = l + gamma_l * updated_l

Sharding: data-parallel over batch (8 batch elements -> 8 NeuronCores).
Each core runs an identical SPMD program on its own batch element.

Host-side preprocessing (pure weight/layout marshaling):
  - LayerNorm gains/biases folded into QKV projection weights/biases.
  - 1/sqrt(head_dim) folded into Wq/bq; gamma residual scales into Wo/bo.
  - Weights passed transposed ([in, out]); activations passed feature-major
    (x^T, [D, T]). The whole device pipeline runs transposed; the host
    transposes outputs back.

Device pipeline per core (all matmuls in fp32r at full PE rate):
  P0  LN standardize (stats via ones-matmul over the partition axis)
  P1  l-side projections: Q^T_l2v, K^T_v2l, V'_v2l      (SBUF resident)
  P2  v-side projections: Q^T_v2l, K^T_l2v, V'_l2v      (DRAM staged)
  P3  v2l attention + out-proj + residual, per 512-query chunk
  P4  l2v attention (streamed over 16 key chunks, 3 head-group passes)

V' tiles pack a ones column per head ([128 keys, 16*65]) so the attention
context matmul also produces the softmax denominator in PSUM row 64.
"""

import numpy as np
from contextlib import ExitStack

import concourse.bass as bass
import concourse.bacc as bacc
import concourse.tile as tile
from concourse import mybir

F32 = mybir.dt.float32
R32 = mybir.dt.float32r

D = 1024
H = 16
HD = 64
TV = 2048
TL = 512
B = 8
NPC = 8          # partition chunks of 128 over D
P = 128
QC = 512         # token/query chunk
EPS = 1e-5

W_NAMES = ["wq_v2l", "wk_v2l", "wv_v2l", "wo_v2l", "wq_l2v", "wk_l2v", "wv_l2v", "wo_l2v"]

_CACHE = {}
PHASES = 5  # bisection aid: how many phases to emit (0..5)


def _mm(nc, out, lhsT, rhs, **kw):
    nc.tensor.matmul(out, lhsT, rhs, **kw)


def _emit(tc: tile.TileContext, io: dict):
    nc = tc.nc
    vT, lT = io["vT"], io["lT"]
    voT, loT = io["voT"], io["loT"]
    biases, bvb = io["biases"], io["bvb"]
    w = {n: io[n] for n in W_NAMES}
    kt_l2v, vp_l2v, qt_d = io["kt_l2v"], io["vp_l2v"], io["qt_v2l"]

    add = mybir.AluOpType.add

    with ExitStack() as octx:
        # ---------------- persistent pools ----------------
        consts = octx.enter_context(tc.tile_pool(name="consts", bufs=1))
        kv = octx.enter_context(tc.tile_pool(name="kv", bufs=16))
        vpv = octx.enter_context(tc.tile_pool(name="vpv", bufs=4))
        bcd = octx.enter_context(tc.tile_pool(name="bcd", bufs=8, space="DRAM"))

        def bcast_via_dram(row_ap, dest_ap, nrows, name):
            """Broadcast an SBUF [1, QC] row to SBUF [nrows, QC] via a DRAM
            bounce (engines cannot shift partitions and SBUF DMA sources
            cannot have stride-0 partition dims; DRAM sources can)."""
            dt = bcd.tile([1, QC], F32, name=f"bcd_{name}", tag="bc", bufs=8)
            nc.sync.dma_start(out=dt, in_=row_ap)
            bc_in = bass.AP(tensor=dt.tensor, offset=dt.offset,
                            ap=[[0, nrows], [1, QC]])
            nc.sync.dma_start(out=dest_ap, in_=bc_in)

        ones_f = consts.tile([P, H], F32)
        nc.vector.memset(ones_f, 1.0)
        ones_col = consts.tile([P, 1], R32)
        nc.vector.tensor_copy(ones_col, ones_f[:, 0:1])
        eps_t = consts.tile([1, 1], F32)
        nc.vector.memset(eps_t, EPS)

        def set_ones_cols(vpt, nh=H):
            # write the per-head ones column (memset cannot produce f32r)
            nc.vector.tensor_copy(
                vpt.rearrange("p (h c) -> p h c", h=nh)[:, :, HD:HD + 1],
                ones_f[:, 0:nh].unsqueeze(2))

        # per-partition bias tiles [128, 8] (column c = bias chunk c)
        # biases dram rows: bq_v2l, bk_v2l, bo_v2l, bq_l2v, bk_l2v, bo_l2v
        bias_t = consts.tile([P, 8, 6], F32)
        for j in range(6):
            nc.sync.dma_start(
                out=bias_t[:, :, j],
                in_=biases[j, :].rearrange("(c p) -> p c", p=P),
            )
        bq_v2l_t = bias_t[:, :, 0]
        bk_v2l_t = bias_t[:, :, 1]
        bo_v2l_t = bias_t[:, :, 2]
        bq_l2v_t = bias_t[:, :, 3]
        bk_l2v_t = bias_t[:, :, 4]
        bo_l2v_t = bias_t[:, :, 5]

        # broadcast V biases [128, 1024] (free-axis layout), per attention
        bvb_t = consts.tile([P, 2, D], F32)
        nc.sync.dma_start(out=bvb_t[:, 0, :], in_=bvb[0])
        nc.sync.dma_start(out=bvb_t[:, 1, :], in_=bvb[1])

        # l-side resident tensors
        qt_l2v = [kv.tile([P, TL], R32, name=f"qt_l2v_{pc}", tag="kv", bufs=16)
                  for pc in range(NPC)]
        kt_v2l = [kv.tile([P, TL], R32, name=f"kt_v2l_{pc}", tag="kv", bufs=16)
                  for pc in range(NPC)]
        vp_v2l = [vpv.tile([P, H * (HD + 1)], R32, name=f"vp_v2l_{k}", tag="vp", bufs=4)
                  for k in range(TL // P)]

        if PHASES < 1:
            return
        # =====================================================
        # zq pool: z^T_v tiles, alive P0..P2
        # =====================================================
        with tc.tile_pool(name="zq", bufs=34) as zq:
            zv = [[None] * (TV // QC) for _ in range(NPC)]
            zl = [None] * NPC

            # ------------- P0 + P1 -------------
            with tc.tile_pool(name="zl_pool", bufs=8) as zlp, \
                 tc.tile_pool(name="xs", bufs=6) as xs, \
                 tc.tile_pool(name="sq", bufs=3) as sqpool, \
                 tc.tile_pool(name="stb", bufs=2) as stb, \
                 tc.tile_pool(name="stbc", bufs=2) as stbc, \
                 tc.tile_pool(name="wp1", bufs=6) as wp, \
                 tc.tile_pool(name="ps01", bufs=4, space="PSUM") as pp01:

                # ---- P0: LayerNorm standardize ----
                for src, T, is_v in ((vT, TV, True), (lT, TL, False)):
                    for t in range(T // QC):
                        ts = slice(t * QC, (t + 1) * QC)
                        sum_ps = pp01.tile([1, QC], F32, name=f"sum_ps_{int(is_v)}_{t}",
                                           tag="st", bufs=4)
                        sq_ps = pp01.tile([1, QC], F32, name=f"sq_ps_{int(is_v)}_{t}",
                                          tag="st", bufs=4)
                        xts = []
                        for pc in range(NPC):
                            xt = xs.tile([P, QC], R32, name=f"x_{int(is_v)}_{t}_{pc}",
                                         tag="x", bufs=11)
                            nc.sync.dma_start(out=xt, in_=src[pc * P:(pc + 1) * P, ts])
                            xts.append(xt)
                            xsq = sqpool.tile([P, QC], R32, name=f"xsq_{int(is_v)}_{t}_{pc}",
                                              tag="xsq", bufs=3)
                            nc.vector.tensor_mul(xsq, xt, xt)
                            _mm(nc, sum_ps, ones_col, xt,
                                start=(pc == 0), stop=(pc == NPC - 1))
                            _mm(nc, sq_ps, ones_col, xsq,
                                start=(pc == 0), stop=(pc == NPC - 1))
                        mu = stb.tile([1, QC], F32, name=f"mu_{int(is_v)}_{t}",
                                      tag="mu", bufs=2)
                        nc.scalar.mul(mu, sum_ps, 1.0 / D)
                        msq = stb.tile([1, QC], F32, name=f"msq_{int(is_v)}_{t}",
                                       tag="msq", bufs=2)
                        nc.scalar.mul(msq, sq_ps, 1.0 / D)
                        var = stb.tile([1, QC], F32, name=f"var_{int(is_v)}_{t}",
                                       tag="var", bufs=2)
                        nc.vector.tensor_mul(var, mu, mu)
                        nc.vector.tensor_sub(var, msq, var)
                        rstd = stb.tile([1, QC], F32, name=f"rstd_{int(is_v)}_{t}",
                                        tag="rstd", bufs=2)
                        nc.scalar.activation(rstd, var,
                                             mybir.ActivationFunctionType.Sqrt,
                                             bias=eps_t)
                        nc.vector.reciprocal(rstd, rstd)
                        mu_b = stbc.tile([P, QC], F32, name=f"mu_b_{int(is_v)}_{t}",
                                         tag="mu_b", bufs=2)
                        bcast_via_dram(mu, mu_b, P, f"mu_{int(is_v)}_{t}")
                        rstd_b = stbc.tile([P, QC], F32, name=f"rstd_b_{int(is_v)}_{t}",
                                           tag="rstd_b", bufs=2)
                        bcast_via_dram(rstd, rstd_b, P, f"rstd_{int(is_v)}_{t}")
                        for pc in range(NPC):
                            xt2 = xts[pc]
                            if is_v:
                                zt = zq.tile([P, QC], R32, name=f"zv_{pc}_{t}",
                                             tag="zq", bufs=34)
                                zv[pc][t] = zt
                            else:
                                zt = zlp.tile([P, QC], R32, name=f"zl_{pc}",
                                              tag="zl", bufs=8)
                                zl[pc] = zt
                            nc.vector.tensor_sub(zt, xt2, mu_b)
                            nc.vector.tensor_mul(zt, zt, rstd_b)

                # ---- P1: l-side projections ----
                if PHASES < 2:
                    return
                def project_T1(wT, bias_col, src, dests, wtag):
                    for og in range(2):
                        ps = [pp01.tile([P, QC], F32, name=f"pt_{wtag}_{og}_{j}",
                                        tag="pp", bufs=4) for j in range(4)]
                        for ic in range(NPC):
                            wt = wp.tile([P, 4 * P], R32, name=f"w_{wtag}_{og}_{ic}",
                                         tag="w", bufs=6)
                            nc.sync.dma_start(
                                out=wt, in_=wT[ic * P:(ic + 1) * P,
                                               og * 4 * P:(og + 1) * 4 * P])
                            for j in range(4):
                                _mm(nc, ps[j], wt[:, j * P:(j + 1) * P], src[ic],
                                    start=(ic == 0), stop=(ic == NPC - 1))
                        for j in range(4):
                            oc = og * 4 + j
                            nc.vector.tensor_scalar_add(dests[oc], ps[j],
                                                        bias_col[:, oc:oc + 1])

                project_T1(w["wq_l2v"], bq_l2v_t, zl, qt_l2v, "ql2v")
                project_T1(w["wk_v2l"], bk_v2l_t, zl, kt_v2l, "kv2l")

                # V'_v2l: natural layout [keys, d] + ones columns
                for k in range(TL // P):
                    set_ones_cols(vp_v2l[k])
                for dh in range(2):
                    ps = [pp01.tile([P, QC], F32, name=f"pv_{dh}_{t}", tag="pp", bufs=4)
                          for t in range(TL // P)]
                    for ic in range(NPC):
                        wt = wp.tile([P, QC], R32, name=f"wv_v2l_{dh}_{ic}",
                                     tag="w", bufs=6)
                        nc.sync.dma_start(
                            out=wt, in_=w["wv_v2l"][ic * P:(ic + 1) * P,
                                                    dh * QC:(dh + 1) * QC])
                        for t in range(TL // P):
                            _mm(nc, ps[t], zl[ic][:, t * P:(t + 1) * P], wt,
                                start=(ic == 0), stop=(ic == NPC - 1))
                    for t in range(TL // P):
                        dst = vp_v2l[t].rearrange("p (h c) -> p h c", h=H)[
                            :, 8 * dh:8 * dh + 8, 0:HD]
                        nc.vector.tensor_add(
                            dst,
                            ps[t].rearrange("p (h c) -> p h c", h=8),
                            bvb_t[:, 0, dh * QC:(dh + 1) * QC].rearrange(
                                "p (h c) -> p h c", h=8))

            # ------------- P2: v-side projections (DRAM staged) -------------
            if PHASES < 3:
                return
            with tc.tile_pool(name="wp2", bufs=6) as wp, \
                 tc.tile_pool(name="stage2", bufs=6) as stage, \
                 tc.tile_pool(name="vstage2", bufs=5) as vstage, \
                 tc.tile_pool(name="pp2", bufs=8, space="PSUM") as pp:

                def project_T2(wT, bias_col, dest_d, wtag):
                    # contraction over zv; output [oc][t] -> DRAM dest_d[oc,:,t*QC:]
                    for og in range(4):
                        ps = [[pp.tile([P, QC], F32, name=f"p_{wtag}_{og}_{t}_{j}",
                                       tag="pp", bufs=8) for j in range(2)]
                              for t in range(TV // QC)]
                        for ic in range(NPC):
                            wt = wp.tile([P, 2 * P], R32, name=f"w_{wtag}_{og}_{ic}",
                                         tag="w", bufs=6)
                            nc.sync.dma_start(
                                out=wt, in_=wT[ic * P:(ic + 1) * P,
                                               og * 2 * P:(og + 1) * 2 * P])
                            for t in range(TV // QC):
                                for j in range(2):
                                    _mm(nc, ps[t][j], wt[:, j * P:(j + 1) * P],
                                        zv[ic][t],
                                        start=(ic == 0), stop=(ic == NPC - 1))
                        for t in range(TV // QC):
                            for j in range(2):
                                oc = og * 2 + j
                                st = stage.tile([P, QC], R32,
                                                name=f"st_{wtag}_{oc}_{t}",
                                                tag="kst", bufs=6)
                                nc.vector.tensor_scalar_add(st, ps[t][j],
                                                            bias_col[:, oc:oc + 1])
                                nc.sync.dma_start(
                                    out=dest_d[oc, :, t * QC:(t + 1) * QC], in_=st)

                project_T2(w["wq_v2l"], bq_v2l_t, qt_d, "qv2l")
                project_T2(w["wk_l2v"], bk_l2v_t, kt_l2v, "kl2v")

                # V'_l2v -> DRAM (natural layout + ones cols)
                # weight half loaded once per dh and kept resident across tcs
                wvt = [[None] * NPC for _ in range(2)]
                for dh in range(2):
                    for ic in range(NPC):
                        wt = wp.tile([P, QC], R32, name=f"wv_l2v_{dh}_{ic}",
                                     tag="wv", bufs=17)
                        nc.sync.dma_start(
                            out=wt, in_=w["wv_l2v"][ic * P:(ic + 1) * P,
                                                    dh * QC:(dh + 1) * QC])
                        wvt[dh][ic] = wt
                for t in range(TV // QC):
                    vst = [vstage.tile([P, H * (HD + 1)], R32, name=f"vst_{t}_{k}",
                                       tag="vst", bufs=5) for k in range(4)]
                    for k in range(4):
                        set_ones_cols(vst[k])
                    for dh in range(2):
                        ps = [pp.tile([P, QC], F32, name=f"pvl_{t}_{dh}_{k}",
                                      tag="pp", bufs=8) for k in range(4)]
                        for ic in range(NPC):
                            for k in range(4):
                                _mm(nc, ps[k], zv[ic][t][:, k * P:(k + 1) * P],
                                    wvt[dh][ic],
                                    start=(ic == 0), stop=(ic == NPC - 1))
                        for k in range(4):
                            dst = vst[k].rearrange("p (h c) -> p h c", h=H)[
                                :, 8 * dh:8 * dh + 8, 0:HD]
                            nc.vector.tensor_add(
                                dst,
                                ps[k].rearrange("p (h c) -> p h c", h=8),
                                bvb_t[:, 1, dh * QC:(dh + 1) * QC].rearrange(
                                    "p (h c) -> p h c", h=8))
                    for k in range(4):
                        nc.sync.dma_start(out=vp_l2v[t * 4 + k], in_=vst[k])

        # =====================================================
        # P3: v2l attention (queries = v tokens, keys = l tokens)
        # =====================================================
        def finish_head(h, ctx_ps, ctxT, scpool, tmppool, tag):
            """Normalize head h's context (PSUM [65, QC], row HD = sumexp) into
            ctxT[h//2] rows [(h%2)*64, ...). Scale fused into the PSUM->SBUF
            move; odd heads hop via SBUF temp + DMA (engines cannot shift
            partitions; DMA cannot read PSUM)."""
            pc = h // 2
            dsc = scpool.tile([HD + 1, QC], F32, name=f"dsc_{tag}_{h}",
                              tag="dsc", bufs=4)
            nc.vector.reciprocal(dsc[HD:HD + 1, :], ctx_ps[HD:HD + 1, :])
            bcast_via_dram(dsc[HD:HD + 1, :], dsc[0:HD, :], HD, f"fh_{tag}_{h}")
            if h % 2 == 0:
                nc.vector.tensor_mul(ctxT[pc][0:HD, :], ctx_ps[0:HD, :], dsc[0:HD, :])
            else:
                tmp = tmppool.tile([HD, QC], R32, name=f"ctmp_{tag}_{h}",
                                   tag="ctmp", bufs=4)
                nc.vector.tensor_mul(tmp, ctx_ps[0:HD, :], dsc[0:HD, :])
                nc.sync.dma_start(out=ctxT[pc][HD:P, :], in_=tmp)

        if PHASES < 4:
            return
        NKC_V2L = TL // P  # 4 key chunks
        with tc.tile_pool(name="wo3", bufs=8) as wop, \
             tc.tile_pool(name="qtc3", bufs=16) as qtcp, \
             tc.tile_pool(name="ppool3", bufs=5) as ppool, \
             tc.tile_pool(name="ctx3", bufs=10) as ctxpool, \
             tc.tile_pool(name="sc3", bufs=4) as scpool, \
             tc.tile_pool(name="tmp3", bufs=4) as tmppool, \
             tc.tile_pool(name="res3", bufs=4) as respool, \
             tc.tile_pool(name="out3", bufs=4) as outpool, \
             tc.tile_pool(name="sps3", bufs=3, space="PSUM") as sps, \
             tc.tile_pool(name="cps3", bufs=2, space="PSUM") as cps, \
             tc.tile_pool(name="pp3", bufs=2, space="PSUM") as pp:

            wo_sb = []
            for ic in range(NPC):
                wt = wop.tile([P, D], R32, name=f"wo_v2l_sb_{ic}", tag="wo", bufs=8)
                nc.sync.dma_start(out=wt, in_=w["wo_v2l"][ic * P:(ic + 1) * P, :])
                wo_sb.append(wt)

            for qc in range(TV // QC):
                qs = slice(qc * QC, (qc + 1) * QC)
                qtc = []
                for pc in range(NPC):
                    qt = qtcp.tile([P, QC], R32, name=f"qtc_{qc}_{pc}",
                                   tag="qtc", bufs=16)
                    nc.sync.dma_start(out=qt, in_=qt_d[pc, :, qs])
                    qtc.append(qt)
                ctxT = [ctxpool.tile([P, QC], R32, name=f"ctxT_{qc}_{pc}",
                                     tag="ctx", bufs=10) for pc in range(NPC)]
                for h in range(H):
                    pc, r0 = h // 2, (h % 2) * HD
                    ctx_ps = cps.tile([HD + 1, QC], F32, name=f"ctx_ps_{qc}_{h}",
                                      tag="cps", bufs=2)
                    for kc in range(NKC_V2L):
                        s_ps = sps.tile([P, QC], F32, name=f"s_ps_{qc}_{h}_{kc}",
                                        tag="sps", bufs=3)
                        _mm(nc, s_ps,
                            kt_v2l[pc][r0:r0 + HD, kc * P:(kc + 1) * P],
                            qtc[pc][r0:r0 + HD, :],
                            start=True, stop=True)
                        p_t = ppool.tile([P, QC], R32, name=f"p_{qc}_{h}_{kc}",
                                         tag="p", bufs=5)
                        nc.scalar.activation(p_t, s_ps,
                                             mybir.ActivationFunctionType.Exp)
                        _mm(nc, ctx_ps,
                            vp_v2l[kc][:, h * (HD + 1):(h + 1) * (HD + 1)],
                            p_t, start=(kc == 0), stop=(kc == NKC_V2L - 1))
                    finish_head(h, ctx_ps, ctxT, scpool, tmppool, f"v2l_{qc}")
                # o-projection + residual
                for og in range(4):
                    ps = [pp.tile([P, QC], F32, name=f"po_{qc}_{og}_{j}",
                                  tag="pp", bufs=2) for j in range(2)]
                    for ic in range(NPC):
                        for j in range(2):
                            oc_ = og * 2 + j
                            _mm(nc, ps[j], wo_sb[ic][:, oc_ * P:(oc_ + 1) * P],
                                ctxT[ic],
                                start=(ic == 0), stop=(ic == NPC - 1))
                    for j in range(2):
                        oc = og * 2 + j
                        vres = respool.tile([P, QC], R32, name=f"vres_{qc}_{oc}",
                                            tag="res", bufs=4)
                        nc.sync.dma_start(out=vres, in_=vT[oc * P:(oc + 1) * P, qs])
                        out_t = outpool.tile([P, QC], F32, name=f"vout_{qc}_{oc}",
                                             tag="o", bufs=4)
                        nc.vector.scalar_tensor_tensor(
                            out=out_t, in0=ps[j], scalar=bo_v2l_t[:, oc:oc + 1],
                            in1=vres, op0=add, op1=add)
                        nc.sync.dma_start(out=voT[oc * P:(oc + 1) * P, qs], in_=out_t)

        if PHASES < 5:
            return
        # =====================================================
        # P4: l2v attention (queries = l tokens, keys = v tokens)
        # =====================================================
        NKC = TV // P  # 16
        groups = [(0, 6), (6, 12), (12, 16)]
        with tc.tile_pool(name="ctx4", bufs=8) as ctxpool, \
             tc.tile_pool(name="sc4", bufs=4) as scpool, \
             tc.tile_pool(name="tmp4", bufs=4) as tmppool:

            ctxT = [ctxpool.tile([P, QC], R32, name=f"ctxT_l2v_{pc}",
                                 tag="ctx", bufs=8) for pc in range(NPC)]
            with tc.tile_pool(name="kt4", bufs=8) as ktp, \
                 tc.tile_pool(name="vp4", bufs=4) as vpp, \
                 tc.tile_pool(name="pl4", bufs=5) as ppool, \
                 tc.tile_pool(name="sps4", bufs=2, space="PSUM") as sps, \
                 tc.tile_pool(name="cps4", bufs=6, space="PSUM") as cps:

                for (h0, h1) in groups:
                    nh = h1 - h0
                    pcs = list(range(h0 // 2, (h1 + 1) // 2))
                    ctx_ps = {h: cps.tile([HD + 1, QC], F32, name=f"ctx_ps_l2v_{h}",
                                          tag="cps", bufs=6) for h in range(h0, h1)}
                    for kc in range(NKC):
                        kt_sb = {}
                        for pc in pcs:
                            kt = ktp.tile([P, P], R32, name=f"kt_sb_{h0}_{kc}_{pc}",
                                          tag="kt", bufs=8)
                            nc.sync.dma_start(out=kt,
                                              in_=kt_l2v[pc, :, kc * P:(kc + 1) * P])
                            kt_sb[pc] = kt
                        vp_sb = vpp.tile([P, nh * (HD + 1)], R32,
                                         name=f"vp_sb_{h0}_{kc}", tag="vps", bufs=4)
                        nc.sync.dma_start(
                            out=vp_sb,
                            in_=vp_l2v[kc][:, h0 * (HD + 1):h1 * (HD + 1)])
                        for h in range(h0, h1):
                            pc, r0 = h // 2, (h % 2) * HD
                            s_ps = sps.tile([P, QC], F32, name=f"s_ps_l2v_{h}_{kc}",
                                            tag="sps", bufs=2)
                            _mm(nc, s_ps,
                                kt_sb[pc][r0:r0 + HD, :],
                                qt_l2v[pc][r0:r0 + HD, :],
                                start=True, stop=True)
                            p_t = ppool.tile([P, QC], R32, name=f"p_l2v_{h}_{kc}",
                                             tag="p", bufs=5)
                            nc.scalar.activation(p_t, s_ps,
                                                 mybir.ActivationFunctionType.Exp)
                            _mm(nc, ctx_ps[h],
                                vp_sb[:, (h - h0) * (HD + 1):(h - h0 + 1) * (HD + 1)],
                                p_t, start=(kc == 0), stop=(kc == NKC - 1))
                    for h in range(h0, h1):
                        finish_head(h, ctx_ps[h], ctxT, scpool, tmppool, "l2v")

            # o-projection + residual
            with tc.tile_pool(name="wp4", bufs=8) as wp, \
                 tc.tile_pool(name="res4", bufs=4) as respool, \
                 tc.tile_pool(name="out4", bufs=4) as outpool, \
                 tc.tile_pool(name="pp4", bufs=2, space="PSUM") as pp:

                wo_sb4 = []
                for ic in range(NPC):
                    wt = wp.tile([P, D], R32, name=f"wo_l2v_sb_{ic}", tag="wo", bufs=8)
                    nc.sync.dma_start(out=wt, in_=w["wo_l2v"][ic * P:(ic + 1) * P, :])
                    wo_sb4.append(wt)
                for og in range(4):
                    ps = [pp.tile([P, QC], F32, name=f"po_l2v_{og}_{j}",
                                  tag="pp", bufs=2) for j in range(2)]
                    for ic in range(NPC):
                        for j in range(2):
                            oc_ = og * 2 + j
                            _mm(nc, ps[j], wo_sb4[ic][:, oc_ * P:(oc_ + 1) * P],
                                ctxT[ic],
                                start=(ic == 0), stop=(ic == NPC - 1))
                    for j in range(2):
                        oc = og * 2 + j
                        lres = respool.tile([P, QC], R32, name=f"lres_{oc}",
                                            tag="res", bufs=4)
                        nc.sync.dma_start(out=lres, in_=lT[oc * P:(oc + 1) * P, :])
                        out_t = outpool.tile([P, QC], F32, name=f"lout_{oc}",
                                             tag="o", bufs=4)
                        nc.vector.scalar_tensor_tensor(
                            out=out_t, in0=ps[j], scalar=bo_l2v_t[:, oc:oc + 1],
                            in1=lres, op0=add, op1=add)
                        nc.sync.dma_start(out=loT[oc * P:(oc + 1) * P, :], in_=out_t)


def build(reps=1):
    """Build the SPMD Bass program (same for every core). Returns nc.

    reps > 1 replicates the whole body (idempotent) inside one NEFF —
    used only for wall-clock timing with dispatch overhead amortized."""
    key = ("nc", reps)
    if key in _CACHE:
        return _CACHE[key]
    nc = bacc.Bacc("TRN2", target_bir_lowering=False, debug=False)
    io = {}
    io["vT"] = nc.declare_dram_parameter("vT", [D, TV], R32, isOutput=False)
    io["lT"] = nc.declare_dram_parameter("lT", [D, TL], R32, isOutput=False)
    for n in W_NAMES:
        io[n] = nc.declare_dram_parameter(n, [D, D], R32, isOutput=False)
    io["biases"] = nc.declare_dram_parameter("biases", [8, D], F32, isOutput=False)
    io["bvb"] = nc.declare_dram_parameter("bvb", [2, P, D], F32, isOutput=False)
    io["voT"] = nc.declare_dram_parameter("voT", [D, TV], F32, isOutput=True)
    io["loT"] = nc.declare_dram_parameter("loT", [D, TL], F32, isOutput=True)
    # DRAM scratch
    io["qt_v2l"] = nc.dram_tensor("qt_v2l", [NPC, P, TV], R32)
    io["kt_l2v"] = nc.dram_tensor("kt_l2v", [NPC, P, TV], R32)
    io["vp_l2v"] = nc.dram_tensor("vp_l2v", [TV // P, P, H * (HD + 1)], R32)

    with tile.TileContext(nc) as tc:
        for _ in range(reps):
            _emit(tc, io)
    nc.compile()
    _CACHE[key] = nc
    return nc


def prepare_in_maps(inputs):
    """Host-side marshaling: fold LN/scale/gamma into weights, transpose."""
    f32 = np.float32
    scale = f32(1.0 / np.sqrt(HD))

    def fold(Wq, bq, Wk, bk, Wv, bv, Wo, bo, g_q, b_q, g_kv, b_kv, gamma):
        Wq = np.asarray(Wq, f32); Wk = np.asarray(Wk, f32)
        Wv = np.asarray(Wv, f32); Wo = np.asarray(Wo, f32)
        bq = np.asarray(bq, f32); bk = np.asarray(bk, f32)
        bv = np.asarray(bv, f32); bo = np.asarray(bo, f32)
        Wq_ = (Wq * g_q[None, :]) * scale
        bq_ = (bq + Wq @ b_q) * scale
        Wk_ = Wk * g_kv[None, :]
        bk_ = bk + Wk @ b_kv
        Wv_ = Wv * g_kv[None, :]
        bv_ = bv + Wv @ b_kv
        Wo_ = gamma[:, None] * Wo
        bo_ = gamma * bo
        return Wq_, bq_, Wk_, bk_, Wv_, bv_, Wo_, bo_

    g_v = np.asarray(inputs["ln_v_g"], f32); b_v = np.asarray(inputs["ln_v_b"], f32)
    g_l = np.asarray(inputs["ln_l_g"], f32); b_l = np.asarray(inputs["ln_l_b"], f32)
    gam_v = np.asarray(inputs["gamma_v"], f32); gam_l = np.asarray(inputs["gamma_l"], f32)

    (Wq1, bq1, Wk1, bk1, Wv1, bv1, Wo1, bo1) = fold(
        inputs["Wq_v2l"], inputs["bq_v2l"], inputs["Wk_v2l"], inputs["bk_v2l"],
        inputs["Wv_v2l"], inputs["bv_v2l"], inputs["Wo_v2l"], inputs["bo_v2l"],
        g_v, b_v, g_l, b_l, gam_v)
    (Wq2, bq2, Wk2, bk2, Wv2, bv2, Wo2, bo2) = fold(
        inputs["Wq_l2v"], inputs["bq_l2v"], inputs["Wk_l2v"], inputs["bk_l2v"],
        inputs["Wv_l2v"], inputs["bv_l2v"], inputs["Wo_l2v"], inputs["bo_l2v"],
        g_l, b_l, g_v, b_v, gam_l)

    wts = {
        "wq_v2l": np.ascontiguousarray(Wq1.T),
        "wk_v2l": np.ascontiguousarray(Wk1.T),
        "wv_v2l": np.ascontiguousarray(Wv1.T),
        "wo_v2l": np.ascontiguousarray(Wo1.T),
        "wq_l2v": np.ascontiguousarray(Wq2.T),
        "wk_l2v": np.ascontiguousarray(Wk2.T),
        "wv_l2v": np.ascontiguousarray(Wv2.T),
        "wo_l2v": np.ascontiguousarray(Wo2.T),
    }
    biases = np.stack([bq1, bk1, bo1, bq2, bk2, bo2,
                       np.zeros(D, f32), np.zeros(D, f32)])
    biases = np.ascontiguousarray(biases.astype(f32))
    bvb = np.stack([np.broadcast_to(bv1, (P, D)), np.broadcast_to(bv2, (P, D))])
    bvb = np.ascontiguousarray(bvb.astype(f32))

    v = np.asarray(inputs["v"], f32)
    l = np.asarray(inputs["l"], f32)
    in_maps = []
    for b in range(B):
        m = dict(wts)
        m["biases"] = biases
        m["bvb"] = bvb
        m["vT"] = np.ascontiguousarray(v[b].T)
        m["lT"] = np.ascontiguousarray(l[b].T)
        in_maps.append(m)
    return in_maps


def kernel(**inputs):
    from concourse.bass_utils import run_bass_kernel_spmd
    nc = build()
    in_maps = prepare_in_maps(inputs)
    res = run_bass_kernel_spmd(nc, in_maps, list(range(B)))
    v_out = np.stack([np.ascontiguousarray(res.results[b]["voT"].T) for b in range(B)])
    l_out = np.stack([np.ascontiguousarray(res.results[b]["loT"].T) for b in range(B)])
    return (v_out, l_out)


# revision 34
# speedup vs baseline: 1.1154x; 1.1154x over previous
"""Trainium2 Bass/Tile kernel for CrossModalMultiHeadAttention.

Reference computation (per batch element b, eval mode):
    v_norm = LN(v) ; l_norm = LN(l)
    updated_v = MHA(q=v_norm, kv=l_norm; W*_v2l)   # v attends to l
    updated_l = MHA(q=l_norm, kv=v_norm; W*_l2v)   # l attends to v
    v_out = v + gamma_v * updated_v
    l_out = l + gamma_l * updated_l

Sharding: data-parallel over batch (8 batch elements -> 8 NeuronCores).
Each core runs an identical SPMD program on its own batch element.

Host-side preprocessing (pure weight/layout marshaling):
  - LayerNorm gains/biases folded into QKV projection weights/biases.
  - 1/sqrt(head_dim) folded into Wq/bq; gamma residual scales into Wo/bo.
  - Weights passed transposed ([in, out]); activations passed feature-major
    (x^T, [D, T]). The whole device pipeline runs transposed; the host
    transposes outputs back.

Device pipeline per core (all matmuls in fp32r at full PE rate):
  P0  LN standardize (stats via ones-matmul over the partition axis)
  P1  l-side projections: Q^T_l2v, K^T_v2l, V'_v2l      (SBUF resident)
  P2  v-side projections: Q^T_v2l, K^T_l2v, V'_l2v      (DRAM staged)
  P3  v2l attention + out-proj + residual, per 512-query chunk
  P4  l2v attention (streamed over 16 key chunks, 3 head-group passes)

V' tiles pack a ones column per head ([128 keys, 16*65]) so the attention
context matmul also produces the softmax denominator in PSUM row 64.
"""

import numpy as np
from contextlib import ExitStack

import concourse.bass as bass
import concourse.bacc as bacc
import concourse.tile as tile
from concourse import mybir

F32 = mybir.dt.float32
R32 = mybir.dt.float32r

D = 1024
H = 16
HD = 64
TV = 2048
TL = 512
B = 8
NPC = 8          # partition chunks of 128 over D
P = 128
QC = 512         # token/query chunk
EPS = 1e-5

W_NAMES = ["wq_v2l", "wk_v2l", "wv_v2l", "wo_v2l", "wq_l2v", "wk_l2v", "wv_l2v", "wo_l2v"]

_CACHE = {}
PHASES = 5  # bisection aid: how many phases to emit (0..5)


def _mm(nc, out, lhsT, rhs, **kw):
    nc.tensor.matmul(out, lhsT, rhs, **kw)


def _emit(tc: tile.TileContext, io: dict):
    nc = tc.nc
    vT, lT = io["vT"], io["lT"]
    voT, loT = io["voT"], io["loT"]
    biases, bvb = io["biases"], io["bvb"]
    w = {n: io[n] for n in W_NAMES}
    kt_l2v, vp_l2v, qt_d = io["kt_l2v"], io["vp_l2v"], io["qt_v2l"]

    add = mybir.AluOpType.add

    with ExitStack() as octx:
        # ---------------- persistent pools ----------------
        consts = octx.enter_context(tc.tile_pool(name="consts", bufs=1))
        kv = octx.enter_context(tc.tile_pool(name="kv", bufs=16))
        vpv = octx.enter_context(tc.tile_pool(name="vpv", bufs=4))
        bcd = octx.enter_context(tc.tile_pool(name="bcd", bufs=8, space="DRAM"))

        def bcast_via_dram(row_ap, dest_ap, nrows, name):
            """Broadcast an SBUF [1, QC] row to SBUF [nrows, QC] via a DRAM
            bounce (engines cannot shift partitions and SBUF DMA sources
            cannot have stride-0 partition dims; DRAM sources can)."""
            dt = bcd.tile([1, QC], F32, name=f"bcd_{name}", tag="bc", bufs=8)
            nc.sync.dma_start(out=dt, in_=row_ap)
            bc_in = bass.AP(tensor=dt.tensor, offset=dt.offset,
                            ap=[[0, nrows], [1, QC]])
            nc.sync.dma_start(out=dest_ap, in_=bc_in)

        ones_f = consts.tile([P, H], F32)
        nc.vector.memset(ones_f, 1.0)
        ones_col = consts.tile([P, 1], R32)
        nc.vector.tensor_copy(ones_col, ones_f[:, 0:1])
        eps_t = consts.tile([1, 1], F32)
        nc.vector.memset(eps_t, EPS)

        def set_ones_cols(vpt, nh=H):
            # write the per-head ones column (memset cannot produce f32r)
            nc.vector.tensor_copy(
                vpt.rearrange("p (h c) -> p h c", h=nh)[:, :, HD:HD + 1],
                ones_f[:, 0:nh].unsqueeze(2))

        # per-partition bias tiles [128, 8] (column c = bias chunk c)
        # biases dram rows: bq_v2l, bk_v2l, bo_v2l, bq_l2v, bk_l2v, bo_l2v
        bias_t = consts.tile([P, 8, 6], F32)
        for j in range(6):
            nc.sync.dma_start(
                out=bias_t[:, :, j],
                in_=biases[j, :].rearrange("(c p) -> p c", p=P),
            )
        bq_v2l_t = bias_t[:, :, 0]
        bk_v2l_t = bias_t[:, :, 1]
        bo_v2l_t = bias_t[:, :, 2]
        bq_l2v_t = bias_t[:, :, 3]
        bk_l2v_t = bias_t[:, :, 4]
        bo_l2v_t = bias_t[:, :, 5]

        # broadcast V biases [128, 1024] (free-axis layout), per attention
        bvb_t = consts.tile([P, 2, D], F32)
        nc.sync.dma_start(out=bvb_t[:, 0, :], in_=bvb[0])
        nc.sync.dma_start(out=bvb_t[:, 1, :], in_=bvb[1])

        # l-side resident tensors
        qt_l2v = [kv.tile([P, TL], R32, name=f"qt_l2v_{pc}", tag="kv", bufs=16)
                  for pc in range(NPC)]
        kt_v2l = [kv.tile([P, TL], R32, name=f"kt_v2l_{pc}", tag="kv", bufs=16)
                  for pc in range(NPC)]
        vp_v2l = [vpv.tile([P, H * (HD + 1)], R32, name=f"vp_v2l_{k}", tag="vp", bufs=4)
                  for k in range(TL // P)]

        if PHASES < 1:
            return
        # =====================================================
        # zq pool: z^T_v tiles, alive P0..P2
        # =====================================================
        with tc.tile_pool(name="zq", bufs=34) as zq:
            zv = [[None] * (TV // QC) for _ in range(NPC)]
            zl = [None] * NPC

            # ------------- P0 + P1 -------------
            with tc.tile_pool(name="zl_pool", bufs=8) as zlp, \
                 tc.tile_pool(name="xs", bufs=12) as xs, \
                 tc.tile_pool(name="sq", bufs=3) as sqpool, \
                 tc.tile_pool(name="stb", bufs=2) as stb, \
                 tc.tile_pool(name="stbc", bufs=2) as stbc, \
                 tc.tile_pool(name="wp1", bufs=6) as wp, \
                 tc.tile_pool(name="ps01", bufs=4, space="PSUM") as pp01:

                # ---- P0: LayerNorm standardize ----
                for src, T, is_v in ((vT, TV, True), (lT, TL, False)):
                    for t in range(T // QC):
                        ts = slice(t * QC, (t + 1) * QC)
                        sum_ps = pp01.tile([1, QC], F32, name=f"sum_ps_{int(is_v)}_{t}",
                                           tag="st", bufs=4)
                        sq_ps = pp01.tile([1, QC], F32, name=f"sq_ps_{int(is_v)}_{t}",
                                          tag="st", bufs=4)
                        xts = []
                        for pc in range(NPC):
                            xt = xs.tile([P, QC], R32, name=f"x_{int(is_v)}_{t}_{pc}",
                                         tag="x", bufs=12)
                            nc.sync.dma_start(out=xt, in_=src[pc * P:(pc + 1) * P, ts])
                            xts.append(xt)
                            xsq = sqpool.tile([P, QC], R32, name=f"xsq_{int(is_v)}_{t}_{pc}",
                                              tag="xsq", bufs=3)
                            nc.scalar.square(xsq, xt)
                            _mm(nc, sum_ps, ones_col, xt,
                                start=(pc == 0), stop=(pc == NPC - 1))
                            _mm(nc, sq_ps, ones_col, xsq,
                                start=(pc == 0), stop=(pc == NPC - 1))
                        mu = stb.tile([1, QC], F32, name=f"mu_{int(is_v)}_{t}",
                                      tag="mu", bufs=2)
                        nc.scalar.mul(mu, sum_ps, 1.0 / D)
                        msq = stb.tile([1, QC], F32, name=f"msq_{int(is_v)}_{t}",
                                       tag="msq", bufs=2)
                        nc.scalar.mul(msq, sq_ps, 1.0 / D)
                        var = stb.tile([1, QC], F32, name=f"var_{int(is_v)}_{t}",
                                       tag="var", bufs=2)
                        nc.vector.tensor_mul(var, mu, mu)
                        nc.vector.tensor_sub(var, msq, var)
                        rstd = stb.tile([1, QC], F32, name=f"rstd_{int(is_v)}_{t}",
                                        tag="rstd", bufs=2)
                        nc.scalar.activation(rstd, var,
                                             mybir.ActivationFunctionType.Sqrt,
                                             bias=eps_t)
                        nc.vector.reciprocal(rstd, rstd)
                        mu_b = stbc.tile([P, QC], F32, name=f"mu_b_{int(is_v)}_{t}",
                                         tag="mu_b", bufs=2)
                        bcast_via_dram(mu, mu_b, P, f"mu_{int(is_v)}_{t}")
                        rstd_b = stbc.tile([P, QC], F32, name=f"rstd_b_{int(is_v)}_{t}",
                                           tag="rstd_b", bufs=2)
                        bcast_via_dram(rstd, rstd_b, P, f"rstd_{int(is_v)}_{t}")
                        for pc in range(NPC):
                            xt2 = xts[pc]
                            if is_v:
                                zt = zq.tile([P, QC], R32, name=f"zv_{pc}_{t}",
                                             tag="zq", bufs=34)
                                zv[pc][t] = zt
                            else:
                                zt = zlp.tile([P, QC], R32, name=f"zl_{pc}",
                                              tag="zl", bufs=8)
                                zl[pc] = zt
                            nc.vector.tensor_sub(zt, xt2, mu_b)
                            nc.vector.tensor_mul(zt, zt, rstd_b)

                # ---- P1: l-side projections ----
                if PHASES < 2:
                    return
                def project_T1(wT, bias_col, src, dests, wtag):
                    for og in range(2):
                        ps = [pp01.tile([P, QC], F32, name=f"pt_{wtag}_{og}_{j}",
                                        tag="pp", bufs=4) for j in range(4)]
                        for ic in range(NPC):
                            wt = wp.tile([P, 4 * P], R32, name=f"w_{wtag}_{og}_{ic}",
                                         tag="w", bufs=6)
                            nc.sync.dma_start(
                                out=wt, in_=wT[ic * P:(ic + 1) * P,
                                               og * 4 * P:(og + 1) * 4 * P])
                            for j in range(4):
                                _mm(nc, ps[j], wt[:, j * P:(j + 1) * P], src[ic],
                                    start=(ic == 0), stop=(ic == NPC - 1))
                        for j in range(4):
                            oc = og * 4 + j
                            nc.vector.tensor_scalar_add(dests[oc], ps[j],
                                                        bias_col[:, oc:oc + 1])

                project_T1(w["wq_l2v"], bq_l2v_t, zl, qt_l2v, "ql2v")
                project_T1(w["wk_v2l"], bk_v2l_t, zl, kt_v2l, "kv2l")

                # V'_v2l: natural layout [keys, d] + ones columns
                for k in range(TL // P):
                    set_ones_cols(vp_v2l[k])
                for dh in range(2):
                    ps = [pp01.tile([P, QC], F32, name=f"pv_{dh}_{t}", tag="pp", bufs=4)
                          for t in range(TL // P)]
                    for ic in range(NPC):
                        wt = wp.tile([P, QC], R32, name=f"wv_v2l_{dh}_{ic}",
                                     tag="w", bufs=6)
                        nc.sync.dma_start(
                            out=wt, in_=w["wv_v2l"][ic * P:(ic + 1) * P,
                                                    dh * QC:(dh + 1) * QC])
                        for t in range(TL // P):
                            _mm(nc, ps[t], zl[ic][:, t * P:(t + 1) * P], wt,
                                start=(ic == 0), stop=(ic == NPC - 1))
                    for t in range(TL // P):
                        dst = vp_v2l[t].rearrange("p (h c) -> p h c", h=H)[
                            :, 8 * dh:8 * dh + 8, 0:HD]
                        nc.vector.tensor_add(
                            dst,
                            ps[t].rearrange("p (h c) -> p h c", h=8),
                            bvb_t[:, 0, dh * QC:(dh + 1) * QC].rearrange(
                                "p (h c) -> p h c", h=8))

            # ------------- P2: v-side projections (DRAM staged) -------------
            if PHASES < 3:
                return
            with tc.tile_pool(name="wp2", bufs=6) as wp, \
                 tc.tile_pool(name="stage2", bufs=6) as stage, \
                 tc.tile_pool(name="vstage2", bufs=5) as vstage, \
                 tc.tile_pool(name="pp2", bufs=8, space="PSUM") as pp:

                def project_T2(wT, bias_col, dest_d, wtag):
                    # contraction over zv; output [oc][t] -> DRAM dest_d[oc,:,t*QC:]
                    for og in range(4):
                        ps = [[pp.tile([P, QC], F32, name=f"p_{wtag}_{og}_{t}_{j}",
                                       tag="pp", bufs=8) for j in range(2)]
                              for t in range(TV // QC)]
                        for ic in range(NPC):
                            wt = wp.tile([P, 2 * P], R32, name=f"w_{wtag}_{og}_{ic}",
                                         tag="w", bufs=6)
                            nc.sync.dma_start(
                                out=wt, in_=wT[ic * P:(ic + 1) * P,
                                               og * 2 * P:(og + 1) * 2 * P])
                            for t in range(TV // QC):
                                for j in range(2):
                                    _mm(nc, ps[t][j], wt[:, j * P:(j + 1) * P],
                                        zv[ic][t],
                                        start=(ic == 0), stop=(ic == NPC - 1))
                        for t in range(TV // QC):
                            for j in range(2):
                                oc = og * 2 + j
                                st = stage.tile([P, QC], R32,
                                                name=f"st_{wtag}_{oc}_{t}",
                                                tag="kst", bufs=6)
                                nc.vector.tensor_scalar_add(st, ps[t][j],
                                                            bias_col[:, oc:oc + 1])
                                nc.sync.dma_start(
                                    out=dest_d[oc, :, t * QC:(t + 1) * QC], in_=st)

                project_T2(w["wq_v2l"], bq_v2l_t, qt_d, "qv2l")
                project_T2(w["wk_l2v"], bk_l2v_t, kt_l2v, "kl2v")

                # V'_l2v -> DRAM (natural layout + ones cols)
                # weight half loaded once per dh and kept resident across tcs
                wvt = [[None] * NPC for _ in range(2)]
                for dh in range(2):
                    for ic in range(NPC):
                        wt = wp.tile([P, QC], R32, name=f"wv_l2v_{dh}_{ic}",
                                     tag="wv", bufs=17)
                        nc.sync.dma_start(
                            out=wt, in_=w["wv_l2v"][ic * P:(ic + 1) * P,
                                                    dh * QC:(dh + 1) * QC])
                        wvt[dh][ic] = wt
                for t in range(TV // QC):
                    vst = [vstage.tile([P, H * (HD + 1)], R32, name=f"vst_{t}_{k}",
                                       tag="vst", bufs=5) for k in range(4)]
                    for k in range(4):
                        set_ones_cols(vst[k])
                    for dh in range(2):
                        ps = [pp.tile([P, QC], F32, name=f"pvl_{t}_{dh}_{k}",
                                      tag="pp", bufs=8) for k in range(4)]
                        for ic in range(NPC):
                            for k in range(4):
                                _mm(nc, ps[k], zv[ic][t][:, k * P:(k + 1) * P],
                                    wvt[dh][ic],
                                    start=(ic == 0), stop=(ic == NPC - 1))
                        for k in range(4):
                            dst = vst[k].rearrange("p (h c) -> p h c", h=H)[
                                :, 8 * dh:8 * dh + 8, 0:HD]
                            nc.vector.tensor_add(
                                dst,
                                ps[k].rearrange("p (h c) -> p h c", h=8),
                                bvb_t[:, 1, dh * QC:(dh + 1) * QC].rearrange(
                                    "p (h c) -> p h c", h=8))
                    for k in range(4):
                        nc.sync.dma_start(out=vp_l2v[t * 4 + k], in_=vst[k])

        # =====================================================
        # P3: v2l attention (queries = v tokens, keys = l tokens)
        # =====================================================
        def finish_head(h, ctx_ps, ctxT, scpool, tmppool, tag):
            """Normalize head h's context (PSUM [65, QC], row HD = sumexp) into
            ctxT[h//2] rows [(h%2)*64, ...). Scale fused into the PSUM->SBUF
            move; odd heads hop via SBUF temp + DMA (engines cannot shift
            partitions; DMA cannot read PSUM)."""
            pc = h // 2
            dsc = scpool.tile([HD + 1, QC], F32, name=f"dsc_{tag}_{h}",
                              tag="dsc", bufs=4)
            nc.vector.reciprocal(dsc[HD:HD + 1, :], ctx_ps[HD:HD + 1, :])
            bcast_via_dram(dsc[HD:HD + 1, :], dsc[0:HD, :], HD, f"fh_{tag}_{h}")
            if h % 2 == 0:
                nc.vector.tensor_mul(ctxT[pc][0:HD, :], ctx_ps[0:HD, :], dsc[0:HD, :])
            else:
                tmp = tmppool.tile([HD, QC], R32, name=f"ctmp_{tag}_{h}",
                                   tag="ctmp", bufs=4)
                nc.vector.tensor_mul(tmp, ctx_ps[0:HD, :], dsc[0:HD, :])
                nc.sync.dma_start(out=ctxT[pc][HD:P, :], in_=tmp)

        if PHASES < 4:
            return
        NKC_V2L = TL // P  # 4 key chunks
        with tc.tile_pool(name="wo3", bufs=8) as wop, \
             tc.tile_pool(name="qtc3", bufs=16) as qtcp, \
             tc.tile_pool(name="ppool3", bufs=5) as ppool, \
             tc.tile_pool(name="ctx3", bufs=10) as ctxpool, \
             tc.tile_pool(name="sc3", bufs=4) as scpool, \
             tc.tile_pool(name="tmp3", bufs=4) as tmppool, \
             tc.tile_pool(name="res3", bufs=4) as respool, \
             tc.tile_pool(name="out3", bufs=4) as outpool, \
             tc.tile_pool(name="sps3", bufs=3, space="PSUM") as sps, \
             tc.tile_pool(name="cps3", bufs=2, space="PSUM") as cps, \
             tc.tile_pool(name="pp3", bufs=2, space="PSUM") as pp:

            wo_sb = []
            for ic in range(NPC):
                wt = wop.tile([P, D], R32, name=f"wo_v2l_sb_{ic}", tag="wo", bufs=8)
                nc.sync.dma_start(out=wt, in_=w["wo_v2l"][ic * P:(ic + 1) * P, :])
                wo_sb.append(wt)

            for qc in range(TV // QC):
                qs = slice(qc * QC, (qc + 1) * QC)
                qtc = []
                for pc in range(NPC):
                    qt = qtcp.tile([P, QC], R32, name=f"qtc_{qc}_{pc}",
                                   tag="qtc", bufs=16)
                    nc.sync.dma_start(out=qt, in_=qt_d[pc, :, qs])
                    qtc.append(qt)
                ctxT = [ctxpool.tile([P, QC], R32, name=f"ctxT_{qc}_{pc}",
                                     tag="ctx", bufs=10) for pc in range(NPC)]
                # heads processed in even/odd pairs, kc-interleaved: the two
                # score matmuls use disjoint PE row groups (base 0 / 64) and
                # run concurrently in the array
                for hp in range(H // 2):
                    hpair = (2 * hp, 2 * hp + 1)
                    pc = hp
                    ctx_ps = {h: cps.tile([HD + 1, QC], F32,
                                          name=f"ctx_ps_{qc}_{h}",
                                          tag="cps", bufs=4) for h in hpair}
                    for kc in range(NKC_V2L):
                        for h in hpair:
                            r0 = (h % 2) * HD
                            s_ps = sps.tile([P, QC], F32,
                                            name=f"s_ps_{qc}_{h}_{kc}",
                                            tag="sps", bufs=2)
                            _mm(nc, s_ps,
                                kt_v2l[pc][r0:r0 + HD, kc * P:(kc + 1) * P],
                                qtc[pc][r0:r0 + HD, :],
                                start=True, stop=True)
                            p_t = ppool.tile([P, QC], R32,
                                             name=f"p_{qc}_{h}_{kc}",
                                             tag="p", bufs=6)
                            nc.scalar.activation(p_t, s_ps,
                                                 mybir.ActivationFunctionType.Exp)
                            _mm(nc, ctx_ps[h],
                                vp_v2l[kc][:, h * (HD + 1):(h + 1) * (HD + 1)],
                                p_t, start=(kc == 0), stop=(kc == NKC_V2L - 1))
                    for h in hpair:
                        finish_head(h, ctx_ps[h], ctxT, scpool, tmppool,
                                    f"v2l_{qc}")
                # o-projection + residual
                for og in range(4):
                    ps = [pp.tile([P, QC], F32, name=f"po_{qc}_{og}_{j}",
                                  tag="pp", bufs=2) for j in range(2)]
                    for ic in range(NPC):
                        for j in range(2):
                            oc_ = og * 2 + j
                            _mm(nc, ps[j], wo_sb[ic][:, oc_ * P:(oc_ + 1) * P],
                                ctxT[ic],
                                start=(ic == 0), stop=(ic == NPC - 1))
                    for j in range(2):
                        oc = og * 2 + j
                        vres = respool.tile([P, QC], R32, name=f"vres_{qc}_{oc}",
                                            tag="res", bufs=4)
                        nc.sync.dma_start(out=vres, in_=vT[oc * P:(oc + 1) * P, qs])
                        out_t = outpool.tile([P, QC], F32, name=f"vout_{qc}_{oc}",
                                             tag="o", bufs=4)
                        nc.vector.scalar_tensor_tensor(
                            out=out_t, in0=ps[j], scalar=bo_v2l_t[:, oc:oc + 1],
                            in1=vres, op0=add, op1=add)
                        nc.sync.dma_start(out=voT[oc * P:(oc + 1) * P, qs], in_=out_t)

        if PHASES < 5:
            return
        # =====================================================
        # P4: l2v attention (queries = l tokens, keys = v tokens)
        # =====================================================
        NKC = TV // P  # 16
        groups = [(0, 6), (6, 12), (12, 16)]
        with tc.tile_pool(name="ctx4", bufs=8) as ctxpool, \
             tc.tile_pool(name="sc4", bufs=4) as scpool, \
             tc.tile_pool(name="tmp4", bufs=4) as tmppool:

            ctxT = [ctxpool.tile([P, QC], R32, name=f"ctxT_l2v_{pc}",
                                 tag="ctx", bufs=8) for pc in range(NPC)]
            with tc.tile_pool(name="kt4", bufs=12) as ktp, \
                 tc.tile_pool(name="vp4", bufs=6) as vpp, \
                 tc.tile_pool(name="pl4", bufs=5) as ppool, \
                 tc.tile_pool(name="sps4", bufs=2, space="PSUM") as sps, \
                 tc.tile_pool(name="cps4", bufs=6, space="PSUM") as cps:

                for (h0, h1) in groups:
                    nh = h1 - h0
                    pcs = list(range(h0 // 2, (h1 + 1) // 2))
                    ctx_ps = {h: cps.tile([HD + 1, QC], F32, name=f"ctx_ps_l2v_{h}",
                                          tag="cps", bufs=6) for h in range(h0, h1)}
                    for kc in range(NKC):
                        kt_sb = {}
                        for pc in pcs:
                            kt = ktp.tile([P, P], R32, name=f"kt_sb_{h0}_{kc}_{pc}",
                                          tag="kt", bufs=12)
                            nc.sync.dma_start(out=kt,
                                              in_=kt_l2v[pc, :, kc * P:(kc + 1) * P])
                            kt_sb[pc] = kt
                        vp_sb = vpp.tile([P, nh * (HD + 1)], R32,
                                         name=f"vp_sb_{h0}_{kc}", tag="vps", bufs=6)
                        nc.sync.dma_start(
                            out=vp_sb,
                            in_=vp_l2v[kc][:, h0 * (HD + 1):h1 * (HD + 1)])
                        for h in range(h0, h1):
                            pc, r0 = h // 2, (h % 2) * HD
                            s_ps = sps.tile([P, QC], F32, name=f"s_ps_l2v_{h}_{kc}",
                                            tag="sps", bufs=2)
                            _mm(nc, s_ps,
                                kt_sb[pc][r0:r0 + HD, :],
                                qt_l2v[pc][r0:r0 + HD, :],
                                start=True, stop=True)
                            p_t = ppool.tile([P, QC], R32, name=f"p_l2v_{h}_{kc}",
                                             tag="p", bufs=6)
                            nc.scalar.activation(p_t, s_ps,
                                                 mybir.ActivationFunctionType.Exp)
                            _mm(nc, ctx_ps[h],
                                vp_sb[:, (h - h0) * (HD + 1):(h - h0 + 1) * (HD + 1)],
                                p_t, start=(kc == 0), stop=(kc == NKC - 1))
                    for h in range(h0, h1):
                        finish_head(h, ctx_ps[h], ctxT, scpool, tmppool, "l2v")

            # o-projection + residual
            with tc.tile_pool(name="wp4", bufs=8) as wp, \
                 tc.tile_pool(name="res4", bufs=4) as respool, \
                 tc.tile_pool(name="out4", bufs=4) as outpool, \
                 tc.tile_pool(name="pp4", bufs=2, space="PSUM") as pp:

                wo_sb4 = []
                for ic in range(NPC):
                    wt = wp.tile([P, D], R32, name=f"wo_l2v_sb_{ic}", tag="wo", bufs=8)
                    nc.sync.dma_start(out=wt, in_=w["wo_l2v"][ic * P:(ic + 1) * P, :])
                    wo_sb4.append(wt)
                for og in range(4):
                    ps = [pp.tile([P, QC], F32, name=f"po_l2v_{og}_{j}",
                                  tag="pp", bufs=2) for j in range(2)]
                    for ic in range(NPC):
                        for j in range(2):
                            oc_ = og * 2 + j
                            _mm(nc, ps[j], wo_sb4[ic][:, oc_ * P:(oc_ + 1) * P],
                                ctxT[ic],
                                start=(ic == 0), stop=(ic == NPC - 1))
                    for j in range(2):
                        oc = og * 2 + j
                        lres = respool.tile([P, QC], R32, name=f"lres_{oc}",
                                            tag="res", bufs=4)
                        nc.sync.dma_start(out=lres, in_=lT[oc * P:(oc + 1) * P, :])
                        out_t = outpool.tile([P, QC], F32, name=f"lout_{oc}",
                                             tag="o", bufs=4)
                        nc.vector.scalar_tensor_tensor(
                            out=out_t, in0=ps[j], scalar=bo_l2v_t[:, oc:oc + 1],
                            in1=lres, op0=add, op1=add)
                        nc.sync.dma_start(out=loT[oc * P:(oc + 1) * P, :], in_=out_t)


def build(reps=1):
    """Build the SPMD Bass program (same for every core). Returns nc.

    reps > 1 replicates the whole body (idempotent) inside one NEFF —
    used only for wall-clock timing with dispatch overhead amortized."""
    key = ("nc", reps)
    if key in _CACHE:
        return _CACHE[key]
    nc = bacc.Bacc("TRN2", target_bir_lowering=False, debug=False)
    io = {}
    io["vT"] = nc.declare_dram_parameter("vT", [D, TV], R32, isOutput=False)
    io["lT"] = nc.declare_dram_parameter("lT", [D, TL], R32, isOutput=False)
    for n in W_NAMES:
        io[n] = nc.declare_dram_parameter(n, [D, D], R32, isOutput=False)
    io["biases"] = nc.declare_dram_parameter("biases", [8, D], F32, isOutput=False)
    io["bvb"] = nc.declare_dram_parameter("bvb", [2, P, D], F32, isOutput=False)
    io["voT"] = nc.declare_dram_parameter("voT", [D, TV], F32, isOutput=True)
    io["loT"] = nc.declare_dram_parameter("loT", [D, TL], F32, isOutput=True)
    # DRAM scratch
    io["qt_v2l"] = nc.dram_tensor("qt_v2l", [NPC, P, TV], R32)
    io["kt_l2v"] = nc.dram_tensor("kt_l2v", [NPC, P, TV], R32)
    io["vp_l2v"] = nc.dram_tensor("vp_l2v", [TV // P, P, H * (HD + 1)], R32)

    with tile.TileContext(nc) as tc:
        for _ in range(reps):
            _emit(tc, io)
    nc.compile()
    _CACHE[key] = nc
    return nc


def prepare_in_maps(inputs):
    """Host-side marshaling: fold LN/scale/gamma into weights, transpose."""
    f32 = np.float32
    scale = f32(1.0 / np.sqrt(HD))

    def fold(Wq, bq, Wk, bk, Wv, bv, Wo, bo, g_q, b_q, g_kv, b_kv, gamma):
        Wq = np.asarray(Wq, f32); Wk = np.asarray(Wk, f32)
        Wv = np.asarray(Wv, f32); Wo = np.asarray(Wo, f32)
        bq = np.asarray(bq, f32); bk = np.asarray(bk, f32)
        bv = np.asarray(bv, f32); bo = np.asarray(bo, f32)
        Wq_ = (Wq * g_q[None, :]) * scale
        bq_ = (bq + Wq @ b_q) * scale
        Wk_ = Wk * g_kv[None, :]
        bk_ = bk + Wk @ b_kv
        Wv_ = Wv * g_kv[None, :]
        bv_ = bv + Wv @ b_kv
        Wo_ = gamma[:, None] * Wo
        bo_ = gamma * bo
        return Wq_, bq_, Wk_, bk_, Wv_, bv_, Wo_, bo_

    g_v = np.asarray(inputs["ln_v_g"], f32); b_v = np.asarray(inputs["ln_v_b"], f32)
    g_l = np.asarray(inputs["ln_l_g"], f32); b_l = np.asarray(inputs["ln_l_b"], f32)
    gam_v = np.asarray(inputs["gamma_v"], f32); gam_l = np.asarray(inputs["gamma_l"], f32)

    (Wq1, bq1, Wk1, bk1, Wv1, bv1, Wo1, bo1) = fold(
        inputs["Wq_v2l"], inputs["bq_v2l"], inputs["Wk_v2l"], inputs["bk_v2l"],
        inputs["Wv_v2l"], inputs["bv_v2l"], inputs["Wo_v2l"], inputs["bo_v2l"],
        g_v, b_v, g_l, b_l, gam_v)
    (Wq2, bq2, Wk2, bk2, Wv2, bv2, Wo2, bo2) = fold(
        inputs["Wq_l2v"], inputs["bq_l2v"], inputs["Wk_l2v"], inputs["bk_l2v"],
        inputs["Wv_l2v"], inputs["bv_l2v"], inputs["Wo_l2v"], inputs["bo_l2v"],
        g_l, b_l, g_v, b_v, gam_l)

    wts = {
        "wq_v2l": np.ascontiguousarray(Wq1.T),
        "wk_v2l": np.ascontiguousarray(Wk1.T),
        "wv_v2l": np.ascontiguousarray(Wv1.T),
        "wo_v2l": np.ascontiguousarray(Wo1.T),
        "wq_l2v": np.ascontiguousarray(Wq2.T),
        "wk_l2v": np.ascontiguousarray(Wk2.T),
        "wv_l2v": np.ascontiguousarray(Wv2.T),
        "wo_l2v": np.ascontiguousarray(Wo2.T),
    }
    biases = np.stack([bq1, bk1, bo1, bq2, bk2, bo2,
                       np.zeros(D, f32), np.zeros(D, f32)])
    biases = np.ascontiguousarray(biases.astype(f32))
    bvb = np.stack([np.broadcast_to(bv1, (P, D)), np.broadcast_to(bv2, (P, D))])
    bvb = np.ascontiguousarray(bvb.astype(f32))

    v = np.asarray(inputs["v"], f32)
    l = np.asarray(inputs["l"], f32)
    in_maps = []
    for b in range(B):
        m = dict(wts)
        m["biases"] = biases
        m["bvb"] = bvb
        m["vT"] = np.ascontiguousarray(v[b].T)
        m["lT"] = np.ascontiguousarray(l[b].T)
        in_maps.append(m)
    return in_maps


def kernel(**inputs):
    from concourse.bass_utils import run_bass_kernel_spmd
    nc = build()
    in_maps = prepare_in_maps(inputs)
    res = run_bass_kernel_spmd(nc, in_maps, list(range(B)))
    v_out = np.stack([np.ascontiguousarray(res.results[b]["voT"].T) for b in range(B)])
    l_out = np.stack([np.ascontiguousarray(res.results[b]["loT"].T) for b in range(B)])
    return (v_out, l_out)
